# revision 9
# baseline (speedup 1.0000x reference)
"""CNN char encoder (conv widths 1/2/3 -> tanh -> max over time -> highway)
as a Bass/Tile kernel for 8 Trainium2 NeuronCores.

Sharding: data-parallel over the 4096 = 32*128 flattened words; 512 words per
core; all weights replicated. Feature-major on chip ([feat_partition, word]);
host transposes back.

v2 layout: two shifted char-pair streams, both dense (d0@0:50, d1@50:100,
zeros above):
  A-quads u=0..9: chars (2u, 2u+1);  B-quads v=0..8: chars (2v+1, 2v+2).
Every conv position is then 1 matmul (w1/w2) or 2 accumulated matmuls (w3),
all K=128 full-rate fp16, and positions sharing a weight fuse pairwise into
N=1024 moving operands (the bf16/fp8 moving-operand limit), halving matmul
count: w1 10, w2 10, w3 20/bank.  Highway: h-branch fp16 (6 matmuls/ot),
t-branch double-pumped fp8 DoubleRow (3 matmuls/ot, Wt pre-scaled x8 with
sigmoid scale=0.125; feat copied to fp8 once per bank).  fp8 for the convs
was measured (numpy sim) at 3e-2 rel err vs the 2e-2 budget - dead; Wt-only
fp8 sims at 1.1e-2.

PSUM drains are the second wall: every conv position's [128,512] fp32 tile
must be read once by ACT or DVE (gpsimd has no PSUM port).  Routes per
[128,1024] unit (2 positions):
  A (ACT): tanh straight off PSUM -> fp16 scratch (~1.0us), then a second
     level fp16 max on DVE (2x mode, ~0.7us) or gpsimd (slower, but a third
     engine) into a running max;
  D (DVE): fp32-from-PSUM tensor_max into an fp16 accumulator (~1.2us, single
     touch; first unit per bank is a 2x-rate tensor_scalar_max copy).
tanh is monotone so D maxes pre-tanh and tanhs once per bank.  Highway h
uses a fused (bias-add, relu) tensor_scalar at 2x from PSUM on DVE; sigmoid
stays on ACT.  ACT table loads for tanh/sigmoid are prewarmed behind the
input DMA.  Output is fp16, upcast on host.
"""

import numpy as np
import ml_dtypes

import concourse.bass as bass
import concourse.tile as tile
from concourse import bacc, mybir
from concourse.bass_utils import run_bass_kernel_spmd

F32 = mybir.dt.float32
FP16 = mybir.dt.float16
FP8 = mybir.dt.float8e4
ACTF = mybir.ActivationFunctionType
ALU = mybir.AluOpType
DR = mybir.MatmulPerfMode.DoubleRow

N_CORES = 8
B, S, L, C = 32, 128, 20, 50
NW = B * S               # 4096 words total
WPC = NW // N_CORES      # 512 words per core
NA = 10                  # A-quads (chars 2u, 2u+1)
NB = 9                   # B-quads (chars 2v+1, 2v+2)
OUT_DIM = 768
NEG = -60000.0           # max identity for fp16 pre-tanh accumulators

# ---- routing knobs ---------------------------------------------------------
# per bank: how many drain units take the ACT (tanh-first) route; the rest
# take the DVE (fp32-max-first) route.  10 units per bank.
ACT_N = {"w1": 8, "w2": 8, "w3_0": 8, "w3_1": 8, "w3_2": 8, "w3_3": 8}
# second-level fp16 maxes of the ACT route: engine cycle pattern
# (gpsimd TENSOR_TENSOR fails the TRN2 walrus opcode-on-engine check: DVE only)
SECOND_PAT = ["v"]  # g=gpsimd, v=vector
# highway epilogue (sub, mul, add) engine per ot
EPI_PAT = ["v", "v", "v", "v", "v", "v"]


def build_nc():
    nc = bacc.Bacc(
        "TRN2", target_bir_lowering=False, debug=False, num_devices=N_CORES
    )

    xa = nc.dram_tensor("xa", [128, NA * WPC], FP16, kind="ExternalInput")
    xb = nc.dram_tensor("xb", [128, NB * WPC], FP16, kind="ExternalInput")
    wconv = nc.dram_tensor("wconv", [128, 15 * 128], FP16, kind="ExternalInput")
    biasp = nc.dram_tensor("biasp", [128, 18], F32, kind="ExternalInput")
    whp = nc.dram_tensor("whp", [128, 36 * 128], FP16, kind="ExternalInput")
    wtp = nc.dram_tensor("wtp", [128, 2 * 18 * 128], FP8, kind="ExternalInput")
    out = nc.dram_tensor("out", [OUT_DIM, WPC], FP16, kind="ExternalOutput")

    with tile.TileContext(nc) as tc:
        with (
            tc.tile_pool(name="singles", bufs=1) as singles,
            tc.tile_pool(name="psum", bufs=3, space="PSUM") as psum,
            tc.tile_pool(name="warmp", bufs=1, space="PSUM") as warmp,
            tc.tile_pool(name="gscr", bufs=4) as gscr,
            tc.tile_pool(name="hwtiles", bufs=2) as hwt,
        ):
            # PE warm-up: junk matmuls on an uninitialized tile into a
            # dedicated PSUM bank nobody reads, issued before any data
            # arrives. Keeps the HAM clock-gate at 8/8 through the ~10us
            # DMA startup so the real matmul stream runs at 2.4GHz from
            # its first instruction.
            wsrc = singles.tile([128, WPC], FP16)
            nc.gpsimd.memset(wsrc, 0.25)
            wpsum = warmp.tile([128, WPC], F32)
            for _ in range(32):
                nc.tensor.matmul(wpsum, wsrc[:, 0:128], wsrc)

            # bias first (tiny) so ACT table warmup starts immediately;
            # inputs chunked (quad-pairs) and split by partition-halves
            # across the sync/scalar HWDGE queues so the first conv units
            # can start as soon as their chunk lands; conv weights (w1/w2
            # slice first) + wt on the gpsimd queue; wh behind the input.
            sb_bias = singles.tile([128, 18], F32)
            nc.sync.dma_start(out=sb_bias, in_=biasp.ap())

            warm = singles.tile([128, 2], FP16)
            nc.scalar.activation(warm[:, 0:1], sb_bias[:, 0:1], ACTF.Tanh)
            nc.scalar.activation(warm[:, 1:2], sb_bias[:, 0:1], ACTF.Sigmoid)

            sb_wc = singles.tile([128, 15 * 128], FP16)
            nc.gpsimd.dma_start(out=sb_wc[:, 0:384], in_=wconv.ap()[:, 0:384])
            xa_ch = []
            for i in range(5):
                t = singles.tile([128, 2 * WPC], FP16, name=f"xa{i}")
                cs = slice(i * 2 * WPC, (i + 1) * 2 * WPC)
                nc.sync.dma_start(out=t[0:64, :], in_=xa.ap()[0:64, cs])
                nc.scalar.dma_start(out=t[64:128, :], in_=xa.ap()[64:128, cs])
                xa_ch.append(t)
            nc.gpsimd.dma_start(out=sb_wc[:, 384:], in_=wconv.ap()[:, 384:])
            xb_ch = []
            for i in range(5):
                w = 2 * WPC if i < 4 else WPC
                t = singles.tile([128, w], FP16, name=f"xb{i}")
                cs = slice(i * 2 * WPC, i * 2 * WPC + w)
                nc.sync.dma_start(out=t[0:64, :], in_=xb.ap()[0:64, cs])
                nc.scalar.dma_start(out=t[64:128, :], in_=xb.ap()[64:128, cs])
                xb_ch.append(t)
            sb_wt = singles.tile([128, 2 * 18 * 128], FP8)
            nc.gpsimd.dma_start(out=sb_wt, in_=wtp.ap())
            sb_wh = singles.tile([128, 36 * 128], FP16)
            nc.scalar.dma_start(out=sb_wh, in_=whp.ap())

            wcv = sb_wc.rearrange("p (i m) -> p i m", m=128)
            wtv = sb_wt.rearrange("p (j t m) -> p j t m", j=2, m=128)

            def xA(u, k):  # k quads starting at u, flattened moving operand
                t = xa_ch[u // 2].rearrange("p (u n) -> p u n", n=WPC)
                off = u % 2
                assert off + k <= 2
                return t[:, off : off + k, :]

            def xB(u, k):
                t = xb_ch[u // 2].rearrange("p (u n) -> p u n", n=WPC)
                off = u % 2
                assert off + k <= 2
                return t[:, off : off + k, :]

            # conv weight tile indices in wcv
            W1E, W1O, W2A = 0, 1, 2
            W3AB = [3 + b for b in range(4)]
            W3CE = [7 + b for b in range(4)]
            W3CO = [11 + b for b in range(4)]

            featall = singles.tile([128, 6, WPC], FP16)
            feat8 = singles.tile([128, 6, WPC], FP8)

            # each unit: list of per-slot matmul groups; a slot is one conv
            # position = one [128,512] PSUM half (moving operand legal max is
            # 512 elements in normal mode)
            def units_w1():
                for u in (0, 2, 4, 6, 8):
                    for w in (W1E, W1O):
                        yield [
                            [(wcv[:, w, :], xA(u, 1))],
                            [(wcv[:, w, :], xA(u + 1, 1))],
                        ]

            def units_w2():
                for u in (0, 2, 4, 6, 8):
                    yield [
                        [(wcv[:, W2A, :], xA(u, 1))],
                        [(wcv[:, W2A, :], xA(u + 1, 1))],
                    ]
                for v in (0, 2, 4, 6):
                    yield [
                        [(wcv[:, W2A, :], xB(v, 1))],
                        [(wcv[:, W2A, :], xB(v + 1, 1))],
                    ]
                yield [[(wcv[:, W2A, :], xB(8, 1))]]

            def units_w3(b):
                ab, ce, co = wcv[:, W3AB[b], :], wcv[:, W3CE[b], :], wcv[:, W3CO[b], :]
                for k in range(4):
                    yield [
                        [(ab, xA(2 * k, 1)), (ce, xA(2 * k + 1, 1))],
                        [(ab, xA(2 * k + 1, 1)), (ce, xA(2 * k + 2, 1))],
                    ]
                    yield [
                        [(ab, xB(2 * k, 1)), (co, xA(2 * k + 1, 1))],
                        [(ab, xB(2 * k + 1, 1)), (co, xA(2 * k + 2, 1))],
                    ]
                yield [[(ab, xA(8, 1)), (ce, xA(9, 1))]]
                yield [[(ab, xB(8, 1)), (co, xA(9, 1))]]

            second_i = 0

            def conv_bank(name, bank, bias_col, units, n_act):
                nonlocal second_i
                units = list(units)
                n = len(units)
                # spread ACT-route picks across the unit list; units 0/1 are
                # N=1024 so both routes initialize full-width accumulators
                step = n / n_act if n_act else 0
                act_set = {int(i * step) for i in range(n_act)} if n_act else set()
                featw = None  # fp16 [128,1024] running max of tanh (A route)
                accw = None   # fp16 [128,1024] running max pre-tanh (D route)
                for i, slots in enumerate(units):
                    ncols = 512 * len(slots)
                    y2 = psum.tile([128, 2, WPC], F32, name="ypsum2", bufs=3)
                    yflat = y2.rearrange("p a b -> p (a b)")[:, :ncols]
                    for si, mms in enumerate(slots):
                        for mi, (lhsT, rhs) in enumerate(mms):
                            nc.tensor.matmul(
                                y2[:, si, :], lhsT, rhs,
                                start=(mi == 0), stop=(mi == len(mms) - 1),
                            )
                    if i in act_set:
                        if featw is None:
                            featw = singles.tile([128, 2 * WPC], FP16,
                                                 name=f"fw_{name}")
                            nc.scalar.activation(
                                featw[:, :ncols], yflat, ACTF.Tanh, bias=bias_col
                            )
                        else:
                            scr = gscr.tile([128, 2 * WPC], FP16,
                                            name="gscr_t", bufs=4)
                            nc.scalar.activation(
                                scr[:, :ncols], yflat, ACTF.Tanh, bias=bias_col
                            )
                            eng = SECOND_PAT[second_i % len(SECOND_PAT)]
                            second_i += 1
                            e = nc.gpsimd if eng == "g" else nc.vector
                            e.tensor_max(
                                featw[:, :ncols], featw[:, :ncols], scr[:, :ncols]
                            )
                    else:
                        if accw is None:
                            accw = singles.tile([128, 2 * WPC], FP16,
                                                name=f"aw_{name}")
                            nc.vector.tensor_scalar_max(
                                accw[:, :ncols], yflat, NEG
                            )
                        else:
                            nc.vector.tensor_max(
                                accw[:, :ncols], accw[:, :ncols], yflat
                            )
                feat_slot = featall[:, bank, :]
                if featw is not None and accw is not None:
                    nc.vector.tensor_max(
                        feat_slot, featw[:, 0:WPC], featw[:, WPC:]
                    )
                    amax = gscr.tile([128, WPC], FP16, name="amax", bufs=2)
                    nc.vector.tensor_max(amax, accw[:, 0:WPC], accw[:, WPC:])
                    mrg = gscr.tile([128, WPC], FP16, name="mrg", bufs=2)
                    nc.scalar.activation(mrg, amax, ACTF.Tanh, bias=bias_col)
                    nc.vector.tensor_max(feat_slot, feat_slot, mrg)
                elif featw is not None:
                    nc.vector.tensor_max(
                        feat_slot, featw[:, 0:WPC], featw[:, WPC:]
                    )
                else:
                    amax = gscr.tile([128, WPC], FP16, name="amax", bufs=2)
                    nc.vector.tensor_max(amax, accw[:, 0:WPC], accw[:, WPC:])
                    nc.scalar.activation(
                        feat_slot, amax, ACTF.Tanh, bias=bias_col
                    )
                nc.vector.tensor_copy(feat8[:, bank, :], feat_slot)

            conv_bank("w1", 0, sb_bias[:, 0:1], units_w1(), ACT_N["w1"])
            conv_bank("w2", 1, sb_bias[:, 1:2], units_w2(), ACT_N["w2"])
            for b in range(4):
                conv_bank(f"w3_{b}", 2 + b, sb_bias[:, 2 + b : 3 + b],
                          units_w3(b), ACT_N[f"w3_{b}"])

            # highway: h = relu(Wh f + bh), t = sig(Wt f + bt),
            # out = t*(h-f) + f, feature-major [128 out-feats, 512 words]
            for ot in range(6):
                hp2 = psum.tile([128, 2, WPC], F32, name="ypsum2", bufs=3)
                for kt in range(6):
                    blk = (ot * 6 + kt) * 128
                    nc.tensor.matmul(
                        hp2[:, 0, :], sb_wh[:, blk : blk + 128],
                        featall[:, kt, :],
                        start=(kt == 0), stop=(kt == 5),
                    )
                for c in range(3):
                    nc.tensor.matmul(
                        hp2[:, 1, :], wtv[:, :, ot * 3 + c, :],
                        feat8[:, 2 * c : 2 * c + 2, :],
                        start=(c == 0), stop=(c == 2),
                        perf_mode=DR,
                    )
                h_sb = hwt.tile([128, WPC], FP16, name="h_sb", bufs=2)
                nc.vector.tensor_scalar_add(
                    h_sb, hp2[:, 0, :], sb_bias[:, 6 + ot : 7 + ot]
                )
                nc.vector.tensor_scalar_max(h_sb, h_sb, 0.0)
                t_sb = hwt.tile([128, WPC], FP16, name="t_sb", bufs=2)
                nc.scalar.activation(
                    t_sb, hp2[:, 1, :], ACTF.Sigmoid,
                    bias=sb_bias[:, 12 + ot : 13 + ot], scale=0.125,
                )
                e = nc.gpsimd if EPI_PAT[ot] == "g" else nc.vector
                out_sb = hwt.tile([128, WPC], FP16, name="out_sb", bufs=2)
                f_slot = featall[:, ot, :]
                e.tensor_sub(h_sb, h_sb, f_slot)
                e.tensor_mul(h_sb, t_sb, h_sb)
                e.tensor_add(out_sb, h_sb, f_slot)
                nc.sync.dma_start(
                    out=out.ap()[ot * 128 : (ot + 1) * 128, :], in_=out_sb
                )

    nc.compile()
    return nc


def pack_inputs(ts10_input, conv_w0, conv_b0, conv_w1, conv_b1, conv_w2,
                conv_b2, wh_w, wh_b, wt_w, wt_b):
    f = np.float32
    h = np.float16
    f8 = ml_dtypes.float8_e4m3

    X = np.ascontiguousarray(ts10_input, dtype=f).reshape(NW, L, C)

    wc = np.zeros((128, 15, 128), f)
    w1t = conv_w0[:, :, 0].T                    # (50, 128)
    wc[0:C, 0] = w1t
    wc[C : 2 * C, 1] = w1t
    wc[0:C, 2] = conv_w1[:, :, 0].T
    wc[C : 2 * C, 2] = conv_w1[:, :, 1].T
    for b in range(4):
        w3 = conv_w2[b * 128 : (b + 1) * 128]   # (128, 50, 3)
        wc[0:C, 3 + b] = w3[:, :, 0].T
        wc[C : 2 * C, 3 + b] = w3[:, :, 1].T
        wc[0:C, 7 + b] = w3[:, :, 2].T
        wc[C : 2 * C, 11 + b] = w3[:, :, 2].T
    wc = wc.reshape(128, 15 * 128).astype(h)

    biasp = np.zeros((128, 18), f)
    biasp[:, 0] = conv_b0
    biasp[:, 1] = conv_b1
    for b in range(4):
        biasp[:, 2 + b] = conv_b2[b * 128 : (b + 1) * 128]
    for ot in range(6):
        biasp[:, 6 + ot] = wh_b[ot * 128 : (ot + 1) * 128]
        biasp[:, 12 + ot] = wt_b[ot * 128 : (ot + 1) * 128]

    whp = np.ascontiguousarray(
        wh_w.reshape(6, 128, 6, 128).transpose(3, 0, 2, 1).reshape(128, 36 * 128)
    ).astype(h)
    # wtp[p, j, ot*3+c, m] = 8*wt_w[ot*128+m, (2c+j)*128+p]
    wt8 = np.clip(wt_w * 8.0, -240, 240).reshape(6, 128, 3, 2, 128)
    wtp = np.ascontiguousarray(
        wt8.transpose(4, 3, 0, 2, 1).reshape(128, 2 * 18 * 128)
    ).astype(f8)

    shared = dict(wconv=wc, biasp=biasp, whp=whp, wtp=wtp)
    in_maps = []
    for c in range(N_CORES):
        Xc = X[c * WPC : (c + 1) * WPC]            # [512, 20, 50]
        xa = np.zeros((128, NA, WPC), f)
        xpair = Xc.reshape(WPC, NA, 2, C).transpose(1, 2, 3, 0)  # [10,2,C,512]
        xa[0:C] = xpair[:, 0].transpose(1, 0, 2)
        xa[C : 2 * C] = xpair[:, 1].transpose(1, 0, 2)
        xb = np.zeros((128, NB, WPC), f)
        xsh = Xc[:, 1:19].reshape(WPC, NB, 2, C).transpose(1, 2, 3, 0)
        xb[0:C] = xsh[:, 0].transpose(1, 0, 2)
        xb[C : 2 * C] = xsh[:, 1].transpose(1, 0, 2)
        in_maps.append(dict(
            xa=xa.reshape(128, NA * WPC).astype(h),
            xb=xb.reshape(128, NB * WPC).astype(h),
            **shared,
        ))
    return in_maps


_NC_CACHE = None


def get_nc():
    global _NC_CACHE
    if _NC_CACHE is None:
        _NC_CACHE = build_nc()
    return _NC_CACHE


def kernel(**inputs):
    in_maps = pack_inputs(**{k: np.asarray(v) for k, v in inputs.items()})
    nc = get_nc()
    res = run_bass_kernel_spmd(nc, in_maps, core_ids=list(range(N_CORES)))
    full = np.empty((NW, OUT_DIM), np.float32)
    for c in range(N_CORES):
        full[c * WPC : (c + 1) * WPC] = res.results[c]["out"].T.astype(np.float32)
    return full.reshape(B, S, OUT_DIM)


# revision 15
# speedup vs baseline: 1.0520x; 1.0520x over previous
"""CNN char encoder (conv widths 1/2/3 -> tanh -> max over time -> highway)
as a Bass/Tile kernel for 8 Trainium2 NeuronCores.

Sharding: data-parallel over the 4096 = 32*128 flattened words; 512 words per
core; all weights replicated. Feature-major on chip ([feat_partition, word]);
host transposes back.

v2 layout: two shifted char-pair streams, both dense (d0@0:50, d1@50:100,
zeros above):
  A-quads u=0..9: chars (2u, 2u+1);  B-quads v=0..8: chars (2v+1, 2v+2).
Every conv position is then 1 matmul (w1/w2) or 2 accumulated matmuls (w3),
all K=128 full-rate fp16, and positions sharing a weight fuse pairwise into
N=1024 moving operands (the bf16/fp8 moving-operand limit), halving matmul
count: w1 10, w2 10, w3 20/bank.  Highway: h-branch fp16 (6 matmuls/ot),
t-branch double-pumped fp8 DoubleRow (3 matmuls/ot, Wt pre-scaled x8 with
sigmoid scale=0.125; feat copied to fp8 once per bank).  fp8 for the convs
was measured (numpy sim) at 3e-2 rel err vs the 2e-2 budget - dead; Wt-only
fp8 sims at 1.1e-2.

PSUM drains are the second wall: every conv position's [128,512] fp32 tile
must be read once by ACT or DVE (gpsimd has no PSUM port).  Routes per
[128,1024] unit (2 positions):
  A (ACT): tanh straight off PSUM -> fp16 scratch (~1.0us), then a second
     level fp16 max on DVE (2x mode, ~0.7us) or gpsimd (slower, but a third
     engine) into a running max;
  D (DVE): fp32-from-PSUM tensor_max into an fp16 accumulator (~1.2us, single
     touch; first unit per bank is a 2x-rate tensor_scalar_max copy).
tanh is monotone so D maxes pre-tanh and tanhs once per bank.  Highway h
uses a fused (bias-add, relu) tensor_scalar at 2x from PSUM on DVE; sigmoid
stays on ACT.  ACT table loads for tanh/sigmoid are prewarmed behind the
input DMA.  Output is fp16, upcast on host.
"""

import numpy as np
import ml_dtypes

import concourse.bass as bass
import concourse.tile as tile
from concourse import bacc, mybir
from concourse.bass_utils import run_bass_kernel_spmd

F32 = mybir.dt.float32
FP16 = mybir.dt.float16
FP8 = mybir.dt.float8e4
ACTF = mybir.ActivationFunctionType
ALU = mybir.AluOpType
DR = mybir.MatmulPerfMode.DoubleRow

N_CORES = 8
B, S, L, C = 32, 128, 20, 50
NW = B * S               # 4096 words total
WPC = NW // N_CORES      # 512 words per core
NA = 10                  # A-quads (chars 2u, 2u+1)
NB = 9                   # B-quads (chars 2v+1, 2v+2)
OUT_DIM = 768
NEG = -60000.0           # max identity for fp16 pre-tanh accumulators

# ---- routing knobs ---------------------------------------------------------
# per bank: how many drain units take the ACT (tanh-first) route; the rest
# take the DVE (fp32-max-first) route.  10 units per bank.
ACT_N = {"w1": 8, "w2": 7, "w3_0": 8, "w3_1": 7, "w3_2": 8, "w3_3": 7}
# second-level fp16 maxes of the ACT route: engine cycle pattern.
# v = DVE tensor_max. (gpsimd TENSOR_TENSOR fails the TRN2 walrus
# opcode-on-engine check, and local DMA accum_op=max is rejected by the
# verifier, so DVE is the only engine for these.)
SECOND_PAT = ["v"]
# highway epilogue (sub, mul, add) engine per ot
EPI_PAT = ["v", "v", "v", "v", "v", "v"]


def build_nc():
    nc = bacc.Bacc(
        "TRN2", target_bir_lowering=False, debug=False, num_devices=N_CORES
    )

    xa = nc.dram_tensor("xa", [128, NA * WPC], FP16, kind="ExternalInput")
    xb = nc.dram_tensor("xb", [128, NB * WPC], FP16, kind="ExternalInput")
    wconv = nc.dram_tensor("wconv", [128, 15 * 128], FP16, kind="ExternalInput")
    biasp = nc.dram_tensor("biasp", [128, 18], F32, kind="ExternalInput")
    whp = nc.dram_tensor("whp", [128, 36 * 128], FP16, kind="ExternalInput")
    wtp = nc.dram_tensor("wtp", [128, 2 * 18 * 128], FP8, kind="ExternalInput")
    out = nc.dram_tensor("out", [OUT_DIM, WPC], FP16, kind="ExternalOutput")

    with tile.TileContext(nc) as tc:
        with (
            tc.tile_pool(name="singles", bufs=1) as singles,
            tc.tile_pool(name="psum", bufs=3, space="PSUM") as psum,
            tc.tile_pool(name="warmp", bufs=1, space="PSUM") as warmp,
            tc.tile_pool(name="gscr", bufs=4) as gscr,
            tc.tile_pool(name="hwtiles", bufs=2) as hwt,
        ):
            # PE warm-up: junk matmuls on an uninitialized tile into a
            # dedicated PSUM bank nobody reads, issued before any data
            # arrives. Keeps the HAM clock-gate at 8/8 through the ~10us
            # DMA startup so the real matmul stream runs at 2.4GHz from
            # its first instruction.
            wsrc = singles.tile([128, WPC], FP16)
            nc.gpsimd.memset(wsrc, 0.25)
            wpsum = warmp.tile([128, WPC], F32)
            for _ in range(16):
                nc.tensor.matmul(wpsum, wsrc[:, 0:128], wsrc)

            # bias first (tiny) so ACT table warmup starts immediately;
            # inputs chunked (quad-pairs) and split by partition-halves
            # across the sync/scalar HWDGE queues so the first conv units
            # can start as soon as their chunk lands; conv weights (w1/w2
            # slice first) + wt on the gpsimd queue; wh behind the input.
            sb_bias = singles.tile([128, 18], F32)
            nc.sync.dma_start(out=sb_bias, in_=biasp.ap())

            warm = singles.tile([128, 2], FP16)
            nc.scalar.activation(warm[:, 0:1], sb_bias[:, 0:1], ACTF.Tanh)
            nc.scalar.activation(warm[:, 1:2], sb_bias[:, 0:1], ACTF.Sigmoid)

            sb_wc = singles.tile([128, 15 * 128], FP16)
            nc.gpsimd.dma_start(out=sb_wc[:, 0:384], in_=wconv.ap()[:, 0:384])
            xa_ch = []
            for i in range(5):
                t = singles.tile([128, 2 * WPC], FP16, name=f"xa{i}")
                cs = slice(i * 2 * WPC, (i + 1) * 2 * WPC)
                nc.sync.dma_start(out=t[0:64, :], in_=xa.ap()[0:64, cs])
                nc.scalar.dma_start(out=t[64:128, :], in_=xa.ap()[64:128, cs])
                xa_ch.append(t)
            nc.gpsimd.dma_start(out=sb_wc[:, 384:], in_=wconv.ap()[:, 384:])
            xb_ch = []
            for i in range(5):
                w = 2 * WPC if i < 4 else WPC
                t = singles.tile([128, w], FP16, name=f"xb{i}")
                cs = slice(i * 2 * WPC, i * 2 * WPC + w)
                nc.sync.dma_start(out=t[0:64, :], in_=xb.ap()[0:64, cs])
                nc.scalar.dma_start(out=t[64:128, :], in_=xb.ap()[64:128, cs])
                xb_ch.append(t)
            sb_wt = singles.tile([128, 2 * 18 * 128], FP8)
            nc.gpsimd.dma_start(out=sb_wt, in_=wtp.ap())
            sb_wh = singles.tile([128, 36 * 128], FP16)
            nc.scalar.dma_start(out=sb_wh, in_=whp.ap())

            wcv = sb_wc.rearrange("p (i m) -> p i m", m=128)
            wtv = sb_wt.rearrange("p (j t m) -> p j t m", j=2, m=128)

            def xA(u, k):  # k quads starting at u, flattened moving operand
                t = xa_ch[u // 2].rearrange("p (u n) -> p u n", n=WPC)
                off = u % 2
                assert off + k <= 2
                return t[:, off : off + k, :]

            def xB(u, k):
                t = xb_ch[u // 2].rearrange("p (u n) -> p u n", n=WPC)
                off = u % 2
                assert off + k <= 2
                return t[:, off : off + k, :]

            # conv weight tile indices in wcv
            W1E, W1O, W2A = 0, 1, 2
            W3AB = [3 + b for b in range(4)]
            W3CE = [7 + b for b in range(4)]
            W3CO = [11 + b for b in range(4)]

            featall = singles.tile([128, 6, WPC], FP16)
            feat8 = singles.tile([128, 6, WPC], FP8)

            # each unit: list of per-slot matmul groups; a slot is one conv
            # position = one [128,512] PSUM half (moving operand legal max is
            # 512 elements in normal mode)
            def units_w1():
                for u in (0, 2, 4, 6, 8):
                    for w in (W1E, W1O):
                        yield [
                            [(wcv[:, w, :], xA(u, 1))],
                            [(wcv[:, w, :], xA(u + 1, 1))],
                        ]

            def units_w2():
                for u in (0, 2, 4, 6, 8):
                    yield [
                        [(wcv[:, W2A, :], xA(u, 1))],
                        [(wcv[:, W2A, :], xA(u + 1, 1))],
                    ]
                for v in (0, 2, 4, 6):
                    yield [
                        [(wcv[:, W2A, :], xB(v, 1))],
                        [(wcv[:, W2A, :], xB(v + 1, 1))],
                    ]
                yield [[(wcv[:, W2A, :], xB(8, 1))]]

            def units_w3(b):
                ab, ce, co = wcv[:, W3AB[b], :], wcv[:, W3CE[b], :], wcv[:, W3CO[b], :]
                for k in range(4):
                    yield [
                        [(ab, xA(2 * k, 1)), (ce, xA(2 * k + 1, 1))],
                        [(ab, xA(2 * k + 1, 1)), (ce, xA(2 * k + 2, 1))],
                    ]
                    yield [
                        [(ab, xB(2 * k, 1)), (co, xA(2 * k + 1, 1))],
                        [(ab, xB(2 * k + 1, 1)), (co, xA(2 * k + 2, 1))],
                    ]
                yield [[(ab, xA(8, 1)), (ce, xA(9, 1))]]
                yield [[(ab, xB(8, 1)), (co, xA(9, 1))]]

            second_i = 0

            def conv_bank(name, bank, bias_col, units, n_act):
                nonlocal second_i
                units = list(units)
                n = len(units)
                # spread ACT-route picks across the unit list; units 0/1 are
                # N=1024 so both routes initialize full-width accumulators
                step = n / n_act if n_act else 0
                act_set = {int(i * step) for i in range(n_act)} if n_act else set()
                featw = None  # fp16 [128,1024] running max of tanh (A route)
                accw = None   # fp16 [128,1024] running max pre-tanh (D route)
                for i, slots in enumerate(units):
                    ncols = 512 * len(slots)
                    y2 = psum.tile([128, 2, WPC], F32, name="ypsum2", bufs=3)
                    yflat = y2.rearrange("p a b -> p (a b)")[:, :ncols]
                    for si, mms in enumerate(slots):
                        for mi, (lhsT, rhs) in enumerate(mms):
                            nc.tensor.matmul(
                                y2[:, si, :], lhsT, rhs,
                                start=(mi == 0), stop=(mi == len(mms) - 1),
                            )
                    if i in act_set:
                        if featw is None:
                            featw = singles.tile([128, 2 * WPC], FP16,
                                                 name=f"fw_{name}")
                            nc.scalar.activation(
                                featw[:, :ncols], yflat, ACTF.Tanh, bias=bias_col
                            )
                        else:
                            scr = gscr.tile([128, 2 * WPC], FP16,
                                            name="gscr_t", bufs=4)
                            nc.scalar.activation(
                                scr[:, :ncols], yflat, ACTF.Tanh, bias=bias_col
                            )
                            eng = SECOND_PAT[second_i % len(SECOND_PAT)]
                            second_i += 1
                            assert eng == "v"
                            nc.vector.tensor_max(
                                featw[:, :ncols], featw[:, :ncols],
                                scr[:, :ncols],
                            )
                    else:
                        if accw is None:
                            accw = singles.tile([128, 2 * WPC], FP16,
                                                name=f"aw_{name}")
                            nc.vector.tensor_scalar_max(
                                accw[:, :ncols], yflat, NEG
                            )
                        else:
                            nc.vector.tensor_max(
                                accw[:, :ncols], accw[:, :ncols], yflat
                            )
                feat_slot = featall[:, bank, :]
                if featw is not None and accw is not None:
                    nc.vector.tensor_max(
                        feat_slot, featw[:, 0:WPC], featw[:, WPC:]
                    )
                    amax = gscr.tile([128, WPC], FP16, name="amax", bufs=2)
                    nc.vector.tensor_max(amax, accw[:, 0:WPC], accw[:, WPC:])
                    mrg = gscr.tile([128, WPC], FP16, name="mrg", bufs=2)
                    nc.scalar.activation(mrg, amax, ACTF.Tanh, bias=bias_col)
                    nc.vector.tensor_max(feat_slot, feat_slot, mrg)
                elif featw is not None:
                    nc.vector.tensor_max(
                        feat_slot, featw[:, 0:WPC], featw[:, WPC:]
                    )
                else:
                    amax = gscr.tile([128, WPC], FP16, name="amax", bufs=2)
                    nc.vector.tensor_max(amax, accw[:, 0:WPC], accw[:, WPC:])
                    nc.scalar.activation(
                        feat_slot, amax, ACTF.Tanh, bias=bias_col
                    )
                nc.vector.tensor_copy(feat8[:, bank, :], feat_slot)

            conv_bank("w1", 0, sb_bias[:, 0:1], units_w1(), ACT_N["w1"])
            conv_bank("w2", 1, sb_bias[:, 1:2], units_w2(), ACT_N["w2"])
            for b in range(4):
                conv_bank(f"w3_{b}", 2 + b, sb_bias[:, 2 + b : 3 + b],
                          units_w3(b), ACT_N[f"w3_{b}"])

            # highway: h = relu(Wh f + bh), t = sig(Wt f + bt),
            # out = t*(h-f) + f, feature-major [128 out-feats, 512 words]
            for ot in range(6):
                hp2 = psum.tile([128, 2, WPC], F32, name="ypsum2", bufs=3)
                for kt in range(6):
                    blk = (ot * 6 + kt) * 128
                    nc.tensor.matmul(
                        hp2[:, 0, :], sb_wh[:, blk : blk + 128],
                        featall[:, kt, :],
                        start=(kt == 0), stop=(kt == 5),
                    )
                for c in range(3):
                    nc.tensor.matmul(
                        hp2[:, 1, :], wtv[:, :, ot * 3 + c, :],
                        feat8[:, 2 * c : 2 * c + 2, :],
                        start=(c == 0), stop=(c == 2),
                        perf_mode=DR,
                    )
                h_sb = hwt.tile([128, WPC], FP16, name="h_sb", bufs=2)
                nc.scalar.activation(
                    h_sb, hp2[:, 0, :], ACTF.Relu,
                    bias=sb_bias[:, 6 + ot : 7 + ot],
                )
                t_sb = hwt.tile([128, WPC], FP16, name="t_sb", bufs=2)
                nc.scalar.activation(
                    t_sb, hp2[:, 1, :], ACTF.Sigmoid,
                    bias=sb_bias[:, 12 + ot : 13 + ot], scale=0.125,
                )
                e = nc.gpsimd if EPI_PAT[ot] == "g" else nc.vector
                out_sb = hwt.tile([128, WPC], FP16, name="out_sb", bufs=2)
                f_slot = featall[:, ot, :]
                e.tensor_sub(h_sb, h_sb, f_slot)
                e.tensor_mul(h_sb, t_sb, h_sb)
                e.tensor_add(out_sb, h_sb, f_slot)
                nc.sync.dma_start(
                    out=out.ap()[ot * 128 : (ot + 1) * 128, :], in_=out_sb
                )

    nc.compile()
    return nc


def pack_inputs(ts10_input, conv_w0, conv_b0, conv_w1, conv_b1, conv_w2,
                conv_b2, wh_w, wh_b, wt_w, wt_b):
    f = np.float32
    h = np.float16
    f8 = ml_dtypes.float8_e4m3

    X = np.ascontiguousarray(ts10_input, dtype=f).reshape(NW, L, C)

    wc = np.zeros((128, 15, 128), f)
    w1t = conv_w0[:, :, 0].T                    # (50, 128)
    wc[0:C, 0] = w1t
    wc[C : 2 * C, 1] = w1t
    wc[0:C, 2] = conv_w1[:, :, 0].T
    wc[C : 2 * C, 2] = conv_w1[:, :, 1].T
    for b in range(4):
        w3 = conv_w2[b * 128 : (b + 1) * 128]   # (128, 50, 3)
        wc[0:C, 3 + b] = w3[:, :, 0].T
        wc[C : 2 * C, 3 + b] = w3[:, :, 1].T
        wc[0:C, 7 + b] = w3[:, :, 2].T
        wc[C : 2 * C, 11 + b] = w3[:, :, 2].T
    wc = wc.reshape(128, 15 * 128).astype(h)

    biasp = np.zeros((128, 18), f)
    biasp[:, 0] = conv_b0
    biasp[:, 1] = conv_b1
    for b in range(4):
        biasp[:, 2 + b] = conv_b2[b * 128 : (b + 1) * 128]
    for ot in range(6):
        biasp[:, 6 + ot] = wh_b[ot * 128 : (ot + 1) * 128]
        biasp[:, 12 + ot] = wt_b[ot * 128 : (ot + 1) * 128]

    whp = np.ascontiguousarray(
        wh_w.reshape(6, 128, 6, 128).transpose(3, 0, 2, 1).reshape(128, 36 * 128)
    ).astype(h)
    # wtp[p, j, ot*3+c, m] = 8*wt_w[ot*128+m, (2c+j)*128+p]
    wt8 = np.clip(wt_w * 8.0, -240, 240).reshape(6, 128, 3, 2, 128)
    wtp = np.ascontiguousarray(
        wt8.transpose(4, 3, 0, 2, 1).reshape(128, 2 * 18 * 128)
    ).astype(f8)

    shared = dict(wconv=wc, biasp=biasp, whp=whp, wtp=wtp)
    in_maps = []
    for c in range(N_CORES):
        Xc = X[c * WPC : (c + 1) * WPC]            # [512, 20, 50]
        xa = np.zeros((128, NA, WPC), f)
        xpair = Xc.reshape(WPC, NA, 2, C).transpose(1, 2, 3, 0)  # [10,2,C,512]
        xa[0:C] = xpair[:, 0].transpose(1, 0, 2)
        xa[C : 2 * C] = xpair[:, 1].transpose(1, 0, 2)
        xb = np.zeros((128, NB, WPC), f)
        xsh = Xc[:, 1:19].reshape(WPC, NB, 2, C).transpose(1, 2, 3, 0)
        xb[0:C] = xsh[:, 0].transpose(1, 0, 2)
        xb[C : 2 * C] = xsh[:, 1].transpose(1, 0, 2)
        in_maps.append(dict(
            xa=xa.reshape(128, NA * WPC).astype(h),
            xb=xb.reshape(128, NB * WPC).astype(h),
            **shared,
        ))
    return in_maps


_NC_CACHE = None


def get_nc():
    global _NC_CACHE
    if _NC_CACHE is None:
        _NC_CACHE = build_nc()
    return _NC_CACHE


def kernel(**inputs):
    in_maps = pack_inputs(**{k: np.asarray(v) for k, v in inputs.items()})
    nc = get_nc()
    res = run_bass_kernel_spmd(nc, in_maps, core_ids=list(range(N_CORES)))
    full = np.empty((NW, OUT_DIM), np.float32)
    for c in range(N_CORES):
        full[c * WPC : (c + 1) * WPC] = res.results[c]["out"].T.astype(np.float32)
    return full.reshape(B, S, OUT_DIM)
